# revision 12
# baseline (speedup 1.0000x reference)
"""Trainium2 Bass kernel for the EnhancedNeuromorphicNetwork HH spiking net.

v4: pair-packed state + hand-authored 2x_1p custom DVE ops.

Strategy (pure batch data-parallel across 8 cores, B=512 -> 64 rows/core):
  - All state lives in SBUF as interleaved bf16 PAIRS, [128, 2048] tiles
    spanning both layers (cols 0:1024 = layer 0, 1024:2048 = layer 1):
      VI = (v, x)   x: iext written per-sweep by ScalarE drains, read by the
                    DVE chain, clobbered with the spike flag by PVRS.
      MB = (m, bm') bm' refreshed per-sweep by a ScalarE exp-activation.
      NB = (n, bn') likewise.
      HX = (h, -)
  - Custom 2x_1p DVE micro-programs (see pairops.py/hh_ops.py) read 4 tensor
    lanes per cycle (lo/hi of both sources) and write a packed pair, so the
    whole HH update runs in 10 fused DVE instructions per layer per step.
  - ScalarE produces the 6 transcendental rate terms (exp/tanh) per layer
    per step, drains PSUM->iext, and densifies the packed spike flags into a
    dense tile for the PE matmuls.
  - Layer 0 runs one timestep ahead of layer 1 so the s0 @ (DT*W1) matmul
    (PE) and the layer-1 elementwise chain overlap; acts for one layer run
    on ScalarE while the DVE chain of the other layer executes.
  - acc += s1 accumulates on the PE into a persistent PSUM bank; the readout
    (acc/T) @ w_out + b_out is computed on-device.
"""
import math
from contextlib import ExitStack

import ml_dtypes
import numpy as np

import concourse.bacc as bacc
import concourse.bass as bass
import concourse.mybir as mybir
import concourse.tile as tile
from concourse.bass_utils import run_bass_kernel_spmd

import hh_ops
from pairops import emit_pair_op

DT = 0.1
B, IN, H0, H1, OUT = 512, 512, 1024, 1024, 128
E0 = int(0.8 * H0)
NCORES = 8
BC = B // NCORES          # 64
KC0 = IN // 128           # 4
NCH = H0 // 128           # 8
HW = NCH * BC             # dense half width per layer (512)
PW = 2 * HW               # pair width per layer (1024)

F32 = mybir.dt.float32
BF16 = mybir.dt.bfloat16
AF = mybir.ActivationFunctionType

RC0 = hh_ops.RC0
RC1 = hh_ops.RC1


def _lo(ap):
    n = ap.shape[1]
    return ap.rearrange("p (n two) -> p n two", two=2)[:, :, 0]


def _hi(ap):
    return ap.rearrange("p (n two) -> p n two", two=2)[:, :, 1]


def _build(T, scal, debug=False):
    v_rest = scal["v_rest"]; v_th = scal["v_threshold"]; v_res = scal["v_reset"]
    gna = scal["g_na_max"]; gk = scal["g_k_max"]; gl = scal["g_leak"]
    ena = scal["e_na"]; ek = scal["e_k"]
    alpha = 1.0 - DT * gl
    beta = DT * gl * v_rest

    ops = hh_ops.register_all()

    nc = bacc.Bacc()
    xT_d = nc.declare_dram_parameter("xT", [IN, BC], F32, isOutput=False)
    w0_d = nc.declare_dram_parameter("w_exc0", [IN, H0], F32, isOutput=False)
    b0r_d = nc.declare_dram_parameter("b0row", [1, H0], F32, isOutput=False)
    w1_d = nc.declare_dram_parameter("w1dt", [H0, H1], BF16, isOutput=False)
    b1r_d = nc.declare_dram_parameter("b1row", [1, H1], BF16, isOutput=False)
    wo_d = nc.declare_dram_parameter("w_out", [H1, OUT], F32, isOutput=False)
    bo_d = nc.declare_dram_parameter("b_out", [128, 1], F32, isOutput=False)
    id_d = nc.declare_dram_parameter("ident", [128, 128], BF16, isOutput=False)
    out_d = nc.declare_dram_parameter("out", [OUT, BC], F32, isOutput=True)
    if debug:
        dbg_d = {f"{nm}{i}": nc.declare_dram_parameter(
                     f"dbg_{nm}{i}", [128, 2 * PW], F32, isOutput=True)
                 for nm in ("vi", "mb", "nb", "hx") for i in range(2)}
        dbga_d = nc.declare_dram_parameter("dbg_acc", [128, HW], F32,
                                           isOutput=True)

    ln = math.log
    with tile.TileContext(nc) as tc, ExitStack() as ctx:
        sb = ctx.enter_context(tc.tile_pool(name="sb", bufs=1))
        pp = ctx.enter_context(tc.tile_pool(name="pp", bufs=1, space="PSUM"))
        pi = ctx.enter_context(tc.tile_pool(name="pi", bufs=2, space="PSUM"))

        # ---- persistent weights ----------------------------------------
        w1sb = sb.tile([128, NCH * H1], BF16)
        w0sb = sb.tile([128, KC0 * H0], F32)
        wosb = sb.tile([128, NCH * OUT], F32)
        xtsb = sb.tile([128, KC0 * BC], F32)
        b0rsb = sb.tile([1, H0], F32)
        b1rsb = sb.tile([1, H1], BF16)
        bosb = sb.tile([128, 1], F32)
        idsb = sb.tile([128, 128], BF16)
        ones_f = sb.tile([1, BC], F32)
        ones_b = sb.tile([1, BC], BF16)

        nc.sync.dma_start(w1sb[:].rearrange("p (c m) -> p c m", c=NCH),
                          w1_d[:].rearrange("(c p) m -> p c m", p=128))
        nc.sync.dma_start(w0sb[:].rearrange("p (c m) -> p c m", c=KC0),
                          w0_d[:].rearrange("(c p) m -> p c m", p=128))
        nc.sync.dma_start(xtsb[:].rearrange("p (c n) -> p c n", c=KC0),
                          xT_d[:].rearrange("(c p) n -> p c n", p=128))
        nc.sync.dma_start(wosb[:].rearrange("p (c o) -> p c o", c=NCH),
                          wo_d[:].rearrange("(c p) o -> p c o", p=128))
        nc.sync.dma_start(b0rsb[:], b0r_d[:])
        nc.sync.dma_start(b1rsb[:], b1r_d[:])
        nc.sync.dma_start(bosb[:], bo_d[:])
        nc.sync.dma_start(idsb[:], id_d[:])

        # ---- state + scratch tiles -------------------------------------
        VI = [sb.tile([128, 2 * PW], BF16, name=f"VI{i}") for i in range(2)]
        MB = [sb.tile([128, 2 * PW], BF16, name=f"MB{i}") for i in range(2)]
        NB = [sb.tile([128, 2 * PW], BF16, name=f"NB{i}") for i in range(2)]
        HX = [sb.tile([128, 2 * PW], BF16, name=f"HX{i}") for i in range(2)]
        EE = sb.tile([128, 2 * PW], BF16)
        AH = sb.tile([128, 2 * PW], BF16)
        AM = sb.tile([128, 2 * PW], BF16)
        AN = sb.tile([128, 2 * PW], BF16)
        MC = sb.tile([128, 2 * PW], BF16)
        IA = sb.tile([128, 2 * PW], BF16)
        Z1 = sb.tile([128, 2 * PW], BF16)
        Z2 = sb.tile([128, 2 * PW], BF16)
        S = sb.tile([128, 2 * HW], BF16)
        RATE = sb.tile([128, HW], F32)
        OUTS = sb.tile([128, BC], F32)
        BIASC = sb.tile([128, 6], F32)

        i0p = pp.tile([128, HW], F32)
        accp = pp.tile([128, HW], F32)
        outp = pp.tile([128, BC], F32)

        nc.vector.memset(ones_f[:], 1.0)
        nc.vector.memset(ones_b[:], 1.0)
        bias_vals = [-4.0,                                  # e1
                     -5.5,                                  # e2
                     -65.0 / 18.0 + ln(4.0 * DT),           # bm'
                     -65.0 / 80.0 + ln(0.125 * DT),         # bn'
                     -65.0 / 20.0 + ln(0.07 * DT),          # ah'
                     35.0 / 20.0]                           # th
        for i, bv in enumerate(bias_vals):
            nc.gpsimd.memset(BIASC[:, i:i + 1], bv)
        bE1, bE2, bBM, bBN, bAH, bTH = (BIASC[:, i:i + 1] for i in range(6))
        zero_bias = 0.0
        for t2 in VI:
            nc.vector.memset(t2[:], 0.0)
            nc.vector.memset(_lo(t2[:]), v_rest)
        for t2 in MB:
            nc.vector.memset(_lo(t2[:]), 0.05)
        for t2 in NB:
            nc.vector.memset(_lo(t2[:]), 0.32)
        for t2 in HX:
            nc.vector.memset(_lo(t2[:]), 0.6)

        # ---- i0p = x @ w0 + T/DT-scaled bias row (persistent PSUM) -----
        for m in range(NCH):
            nc.tensor.matmul(i0p[:, m * BC:(m + 1) * BC],
                             b0rsb[0:1, m * 128:(m + 1) * 128],
                             ones_f[0:1, :], start=True, stop=False)
            for c in range(KC0):
                nc.tensor.matmul(
                    i0p[:, m * BC:(m + 1) * BC],
                    w0sb[:, c * H0 + m * 128: c * H0 + (m + 1) * 128],
                    xtsb[:, c * BC:(c + 1) * BC],
                    start=False, stop=(c == KC0 - 1))

        # layer slices of pair tiles: L0 = [0:PW], L1 = [PW:2PW]
        def pslice(t2, layer):
            return t2[:, layer * PW:(layer + 1) * PW]

        OPS = ops

        def acts(layer, p):
            """ScalarE rate activations for one layer-half."""
            vv = _lo(pslice(VI[p][:], layer))
            nc.scalar.activation(_lo(pslice(EE[:], layer)), vv, AF.Exp,
                                 bias=bE1, scale=-0.1)
            nc.scalar.activation(_hi(pslice(EE[:], layer)), vv, AF.Exp,
                                 bias=bE2, scale=-0.1)
            nc.scalar.activation(_hi(pslice(MB[p][:], layer)), vv, AF.Exp,
                                 bias=bBM, scale=-1.0 / 18.0)
            nc.scalar.activation(_hi(pslice(NB[p][:], layer)), vv, AF.Exp,
                                 bias=bBN, scale=-1.0 / 80.0)
            nc.scalar.activation(_lo(pslice(AH[:], layer)), vv, AF.Exp,
                                 bias=bAH, scale=-1.0 / 20.0)
            nc.scalar.activation(_hi(pslice(AH[:], layer)), vv, AF.Tanh,
                                 bias=bTH, scale=1.0 / 20.0)

        def chain(layer, p, q):
            """The 10 fused DVE ops for one layer-half, state p -> q."""
            vi_p, vi_q = pslice(VI[p][:], layer), pslice(VI[q][:], layer)
            mb_p, mb_q = pslice(MB[p][:], layer), pslice(MB[q][:], layer)
            nb_p, nb_q = pslice(NB[p][:], layer), pslice(NB[q][:], layer)
            hx_p, hx_q = pslice(HX[p][:], layer), pslice(HX[q][:], layer)
            ee, ah = pslice(EE[:], layer), pslice(AH[:], layer)
            am, an = pslice(AM[:], layer), pslice(AN[:], layer)
            mc, ia = pslice(MC[:], layer), pslice(IA[:], layer)
            z1, z2 = pslice(Z1[:], layer), pslice(Z2[:], layer)

            sK1 = math.sqrt(0.1 * DT)
            emit_pair_op(nc, OPS["HH4_PF_AM"], out=am, in0=vi_p, in1=ee,
                         s0=RC0 * sK1, s1=RC1 * sK1, imm2=40.0)
            sK2 = math.sqrt(0.01 * DT)
            emit_pair_op(nc, OPS["HH4_PF_AN"], out=an, in0=vi_p, in1=ee,
                         s0=RC0 * sK2, s1=RC1 * sK2, imm2=55.0)
            emit_pair_op(nc, OPS["HH4_GM"], out=mb_q, in0=am, in1=mb_p)
            emit_pair_op(nc, OPS["HH4_GH"], out=hx_q, in0=ah, in1=hx_p,
                         imm2=DT / 2.0)
            emit_pair_op(nc, OPS["HH4_GN"], out=nb_q, in0=an, in1=nb_p)
            emit_pair_op(nc, OPS["HH4_MH3"], out=mc, in0=mb_q, in1=hx_q)
            emit_pair_op(nc, OPS["HH4_INA2"], out=ia, in0=mc, in1=vi_p,
                         s0=ena, s1=gna * DT)
            emit_pair_op(nc, OPS["HH4_PIK"], out=z1, in0=nb_q, in1=vi_p,
                         s0=ek, s1=gk * DT)
            emit_pair_op(nc, OPS["HH4_OPZ"], out=z2, in0=z1, in1=ia)
            emit_pair_op(nc, OPS["HH4_PVRS"], out=vi_q, in0=z2, in1=vi_p,
                         s0=alpha, s1=v_th, imm2=v_res)

        # ---- the T+1 sweeps --------------------------------------------
        for k in range(T + 1):
            p, q = k % 2, (k + 1) % 2
            i1p_k = (pi.tile([128, HW], F32, tag="i1p", name=f"i1p{k}")
                     if k < T else None)

            if k < T:
                # ---- layer-0 phase: step k ----
                acts(0, p)
                # iext0 refresh (clobbered by previous PVRS's spike flag)
                nc.scalar.activation(_hi(pslice(VI[p][:], 0)), i0p[:],
                                     AF.Identity, bias=0.0, scale=DT / T)
                chain(0, p, q)
                # densify s0(k)
                nc.scalar.activation(S[:, 0:HW], _hi(pslice(VI[q][:], 0)),
                                     AF.Copy)
                # i1(k) = s0(k) @ (DT*W1) + (DT*b1 + beta)
                for m in range(NCH):
                    nc.tensor.matmul(i1p_k[:, m * BC:(m + 1) * BC],
                                     b1rsb[0:1, m * 128:(m + 1) * 128],
                                     ones_b[0:1, :], start=True, stop=False)
                    for c in range(NCH):
                        nc.tensor.matmul(
                            i1p_k[:, m * BC:(m + 1) * BC],
                            w1sb[:, c * H1 + m * 128: c * H1 + (m + 1) * 128],
                            S[:, c * BC:(c + 1) * BC],
                            start=False, stop=(c == NCH - 1))

            if k >= 1:
                # ---- layer-1 phase: step k-1 ----
                acts(1, p)
                nc.scalar.activation(_hi(pslice(VI[p][:], 1)), i1p_prev[:],
                                     AF.Identity, bias=0.0, scale=1.0)
                chain(1, p, q)
                nc.scalar.activation(S[:, HW:2 * HW], _hi(pslice(VI[q][:], 1)),
                                     AF.Copy)
                nc.tensor.matmul(accp[:], idsb[:], S[:, HW:2 * HW],
                                 start=(k == 1), stop=(k == T),
                                 skip_group_check=True)
            i1p_prev = i1p_k

        # ---- readout: (acc/T) @ w_out + b_out --------------------------
        nc.scalar.activation(RATE[:], accp[:], AF.Identity, bias=0.0,
                             scale=1.0 / T)
        for c in range(NCH):
            nc.tensor.matmul(outp[:],
                             wosb[:, c * OUT:(c + 1) * OUT],
                             RATE[:, c * BC:(c + 1) * BC],
                             start=(c == 0), stop=(c == NCH - 1))
        nc.scalar.activation(OUTS[:], outp[:], AF.Identity,
                             bias=bosb[:, 0:1], scale=1.0)
        nc.sync.dma_start(out_d[:], OUTS[:])

        if debug:
            for nm, tl in (("vi", VI), ("mb", MB), ("nb", NB), ("hx", HX)):
                for i in range(2):
                    DBG = sb.tile([128, 2 * PW], F32, name=f"DBG{nm}{i}")
                    nc.vector.tensor_copy(DBG[:], tl[i][:])
                    nc.sync.dma_start(dbg_d[f"{nm}{i}"][:], DBG[:])
            DBGA = sb.tile([128, HW], F32)
            nc.vector.tensor_copy(DBGA[:], accp[:])
            nc.sync.dma_start(dbga_d[:], DBGA[:])
    nc.compile()
    return nc


_NC_CACHE = {}


def _get_nc(T, scal):
    key = (T, tuple(sorted(scal.items())))
    if key not in _NC_CACHE:
        _NC_CACHE[key] = _build(T, scal)
    return _NC_CACHE[key]


def _make_in_maps(inputs, T, scal):
    gl = scal["g_leak"]; v_rest = scal["v_rest"]
    beta = DT * gl * v_rest

    x = np.asarray(inputs["x"], np.float32)
    w_exc0 = np.ascontiguousarray(np.asarray(inputs["w_exc0"], np.float32))
    W1 = np.concatenate([np.asarray(inputs["w_exc1"], np.float32),
                         -np.asarray(inputs["w_inh1"], np.float32)], axis=0)
    w1dt = (DT * W1).astype(ml_dtypes.bfloat16)
    # i0p = x @ w0 + row, drained with scale DT/T: row = (DT*b0+beta)*T/DT
    b0row = ((DT * np.asarray(inputs["b_exc0"], np.float32) + beta)
             * (T / DT)).reshape(1, H0).astype(np.float32)
    b1row = (DT * (np.asarray(inputs["b_exc1"], np.float32)
                   - np.asarray(inputs["b_inh1"], np.float32)) + beta
             ).reshape(1, H1).astype(ml_dtypes.bfloat16)
    w_out = np.ascontiguousarray(np.asarray(inputs["w_out"], np.float32))
    b_out = np.asarray(inputs["b_out"], np.float32).reshape(128, 1)
    ident = np.eye(128, dtype=ml_dtypes.bfloat16)

    in_maps = []
    for c in range(NCORES):
        xT = np.ascontiguousarray(x[c * BC:(c + 1) * BC, :].T)
        in_maps.append({
            "xT": xT, "w_exc0": w_exc0, "b0row": b0row, "w1dt": w1dt,
            "b1row": b1row, "w_out": w_out, "b_out": b_out, "ident": ident,
        })
    return in_maps


def kernel(**inputs):
    T = int(np.asarray(inputs["timesteps"]))
    scal = {k: float(np.asarray(inputs[k])) for k in
            ("v_rest", "v_threshold", "v_reset", "g_na_max", "g_k_max",
             "g_leak", "e_na", "e_k")}
    nc = _get_nc(T, scal)
    in_maps = _make_in_maps(inputs, T, scal)
    res = run_bass_kernel_spmd(nc, in_maps, core_ids=list(range(NCORES)))
    out = np.empty((B, OUT), np.float32)
    for c in range(NCORES):
        out[c * BC:(c + 1) * BC, :] = res.results[c]["out"].T
    return out
